# revision 1
# baseline (speedup 1.0000x reference)
"""Trainium2 Bass kernel for nn_Decoder (2-layer bidirectional LSTM decoder,
autoregressive argmax feedback, T=512 steps, B=128, H=1024, V=64).

Strategy: 8-way tensor parallel over the 4H gate dimension. Each core holds a
512-wide slice of every gate projection (re-ordered [i,f,o,g] so activations
fuse), keeps the LSTM recurrence state resident, and exchanges the 128-row
h-slices it owns via two AllGathers per step. Matmuls run as fp32r (TF32) with
the h-state (transposed) as the stationary operand and the weight slice as the
512-wide moving operand. Logits are computed as per-core partials, summed
after the second AllGather, and the argmax feeds the next step on-device.
"""

import os
import sys

import numpy as np

sys.path.insert(0, "/opt/trn_rl_repo")

import concourse.bass as bass  # noqa: E402
import concourse.mybir as mybir  # noqa: E402
import concourse.tile as tile  # noqa: E402
from concourse import bacc  # noqa: E402
from concourse import bass_utils  # noqa: E402
from concourse.masks import make_identity  # noqa: E402

H = 1024
V = 64
B = 128
NCORES = 8
MASK_IDX = 4.0
KEEP_IDX = 3
T_STEPS = int(os.environ.get("DEC_T", "512"))
CHUNK = int(os.environ.get("DEC_CHUNK", "16"))
MM_DT = mybir.dt.float32r if os.environ.get("DEC_MMDT", "fp32r") == "fp32r" else mybir.dt.float32
F32 = mybir.dt.float32
MMD = MM_DT
AF = mybir.ActivationFunctionType
ALU = mybir.AluOpType

# gate blocks packed per-core as [i, f, o, g] (torch order in rows is i,f,g,o)
GBASE = [0, H, 3 * H, 2 * H]


def tf32_round(x):
    if MM_DT == F32:
        return np.asarray(x, np.float32)
    xi = np.asarray(x, np.float32).view(np.uint32)
    xi = (xi + np.uint32(1 << 12)) & np.uint32(0xFFFFE000)
    return xi.view(np.float32)


def build(T=T_STEPS):
    nc = bacc.Bacc("TRN2", num_devices=NCORES)
    RG = [list(range(NCORES))]

    din = dict(kind="ExternalInput")
    w0T = nc.dram_tensor("w0T", [2, 128, 8, 512], MMD, **din)
    w0aug = nc.dram_tensor("w0aug", [2, 2, 512], MMD, **din)
    w1iT = nc.dram_tensor("w1iT", [2, 128, 16, 512], MMD, **din)
    w1hT = nc.dram_tensor("w1hT", [2, 128, 8, 512], MMD, **din)
    b1row = nc.dram_tensor("b1row", [2, 1, 512], MMD, **din)
    linTc = nc.dram_tensor("linTc", [128, 2, 64], MMD, **din)
    linb = nc.dram_tensor("linb", [128, 64], F32, **din)
    iotam = nc.dram_tensor("iotam", [128, 64], F32, **din)
    notkeep = nc.dram_tensor("notkeep", [128, 64], F32, **din)
    hT0 = nc.dram_tensor("hT0", [4, 128, 8, 128], MMD, **din)
    c0s = nc.dram_tensor("c0s", [4, 128, 128], F32, **din)
    onesrow = nc.dram_tensor("onesrow", [1, 128], MMD, **din)
    x0row = nc.dram_tensor("x0row", [1, 128], MMD, **din)
    flag0 = nc.dram_tensor("flag0", [128, 1], F32, **din)
    hT_f = nc.dram_tensor("hT_f", [4, 128, 8, 128], MMD, kind="ExternalOutput")
    c_f = nc.dram_tensor("c_f", [4, 128, 128], F32, kind="ExternalOutput")
    flag_f = nc.dram_tensor("flag_f", [128, 1], F32, kind="ExternalOutput")
    idx_f = nc.dram_tensor("idx_f", [128, 1], F32, kind="ExternalOutput")
    y = nc.dram_tensor("y", [B, T, V], F32, kind="ExternalOutput")

    with tile.TileContext(nc) as tc:
        import contextlib

        ctx = contextlib.ExitStack()
        with ctx:
            wp = ctx.enter_context(tc.tile_pool(name="weights", bufs=1))
            hp = ctx.enter_context(tc.tile_pool(name="hstate", bufs=2))
            cp = ctx.enter_context(tc.tile_pool(name="cstate", bufs=2))
            gp = ctx.enter_context(tc.tile_pool(name="gact", bufs=1))
            ewp = ctx.enter_context(tc.tile_pool(name="ew", bufs=2))
            sp = ctx.enter_context(tc.tile_pool(name="send", bufs=2))
            ap_ = ctx.enter_context(tc.tile_pool(name="amax", bufs=2))
            yp = ctx.enter_context(tc.tile_pool(name="ybuf", bufs=2))
            pg = ctx.enter_context(tc.tile_pool(name="pgates", bufs=1, space="PSUM"))
            pt = ctx.enter_context(tc.tile_pool(name="ptrans", bufs=2, space="PSUM"))
            px = ctx.enter_context(tc.tile_pool(name="pmisc", bufs=1, space="PSUM"))
            dp = ctx.enter_context(tc.tile_pool(name="dram", bufs=2, space="DRAM"))

            # ---- load weights + constants (once) ----
            w0_sb, w0a_sb, w1i_sb, w1h_sb, b1_sb = [], [], [], [], []
            for d in range(2):
                t_ = wp.tile([128, 8, 512], MMD, tag=f"w0_{d}")
                nc.sync.dma_start(out=t_[:], in_=w0T[d])
                w0_sb.append(t_)
                tb = wp.tile([1, 512], MMD, tag=f"w0b_{d}")
                nc.sync.dma_start(out=tb[:], in_=w0aug[d, 1:2])
                tx = wp.tile([1, 512], MMD, tag=f"w0x_{d}")
                nc.sync.dma_start(out=tx[:], in_=w0aug[d, 0:1])
                w0a_sb.append((tx, tb))
                t_ = wp.tile([128, 16, 512], MMD, tag=f"w1i_{d}")
                nc.sync.dma_start(out=t_[:], in_=w1iT[d])
                w1i_sb.append(t_)
                t_ = wp.tile([128, 8, 512], MMD, tag=f"w1h_{d}")
                nc.sync.dma_start(out=t_[:], in_=w1hT[d])
                w1h_sb.append(t_)
                t_ = wp.tile([1, 512], MMD, tag=f"b1_{d}")
                nc.sync.dma_start(out=t_[:], in_=b1row[d])
                b1_sb.append(t_)
            lin_sb = wp.tile([128, 2, 64], MMD, tag="lin")
            nc.sync.dma_start(out=lin_sb[:], in_=linTc[:])
            linb_sb = wp.tile([128, 64], F32, tag="linb")
            nc.sync.dma_start(out=linb_sb[:], in_=linb[:])
            iot_sb = wp.tile([128, 64], F32, tag="iot")
            nc.sync.dma_start(out=iot_sb[:], in_=iotam[:])
            nk_sb = wp.tile([128, 64], F32, tag="nk")
            nc.sync.dma_start(out=nk_sb[:], in_=notkeep[:])
            ident = wp.tile([128, 128], F32, tag="ident")
            make_identity(nc, ident[:])
            ones = wp.tile([1, 128], MMD, tag="ones")
            nc.sync.dma_start(out=ones[:], in_=onesrow[:])

            # ---- initial state ----
            h_prev = []
            for cell in range(4):
                t_ = hp.tile([128, 8, 128], MMD, tag=f"h{cell}")
                nc.sync.dma_start(out=t_[:], in_=hT0[cell])
                h_prev.append(t_)
            c_prev = []
            for cell in range(4):
                t_ = cp.tile([128, 128], F32, tag=f"c{cell}")
                nc.sync.dma_start(out=t_[:], in_=c0s[cell])
                c_prev.append(t_)
            flag_prev = ap_.tile([128, 1], F32, tag="flag")
            nc.sync.dma_start(out=flag_prev[:], in_=flag0[:])
            x_row = ap_.tile([1, 128], MMD, tag="xrow")
            nc.sync.dma_start(out=x_row[:], in_=x0row[:])

            idx_prev = None
            ybuf = None

            for t in range(T):
                # -- 1) L0 gate partials: bias + w_hh0 (run during prev AG_B window)
                g0 = []
                for d in range(2):
                    g = pg.tile([128, 512], F32, tag=f"g0{d}")
                    nc.tensor.matmul(g[:], (ones[:]), (w0a_sb[d][1][:]),
                                     start=True, stop=False)
                    for k in range(8):
                        nc.tensor.matmul(g[:], (h_prev[d][:, k, :]),
                                         (w0_sb[d][:, k, :]),
                                         start=False, stop=False)
                    g0.append(g)
                # -- 3) x transpose (prev step's argmax -> row layout)
                if idx_prev is not None:
                    x_ps = px.tile([1, 128], F32, tag="xps")
                    nc.tensor.transpose(x_ps[:], idx_prev[:], ident[:])
                    x_row = ap_.tile([1, 128], MMD, tag="xrow")
                    nc.vector.tensor_copy(x_row[:], x_ps[:])
                # -- 4) close L0 gates with x contribution
                for d in range(2):
                    nc.tensor.matmul(g0[d][:], (x_row[:]),
                                     (w0a_sb[d][0][:]),
                                     start=False, stop=True)
                # -- 5) L0 elementwise + transpose own slice
                sendA = sp.tile([128, 256], MMD, tag="sendA")
                agA_in = dp.tile([128, 256], MMD, tag="agAi")
                c_new, h_new = [None] * 4, [None] * 4
                for d in range(2):
                    a = gp.tile([128, 512], F32, tag=f"a{d}")
                    nc.scalar.activation(a[:, 0:384], g0[d][:, 0:384], AF.Sigmoid)
                    nc.scalar.activation(a[:, 384:512], g0[d][:, 384:512], AF.Tanh)
                    t1 = ewp.tile([128, 128], F32, tag="t1")
                    nc.vector.tensor_mul(t1[:], a[:, 128:256], c_prev[d][:])
                    t2 = ewp.tile([128, 128], F32, tag="t2")
                    nc.vector.tensor_mul(t2[:], a[:, 0:128], a[:, 384:512])
                    cn = cp.tile([128, 128], F32, tag=f"c{d}")
                    nc.vector.tensor_add(cn[:], t1[:], t2[:])
                    tc2 = ewp.tile([128, 128], F32, tag="tc2")
                    nc.scalar.activation(tc2[:], cn[:], AF.Tanh)
                    h2 = gp.tile([128, 128], F32, tag=f"h2_{d}")
                    nc.vector.tensor_mul(h2[:], a[:, 256:384], tc2[:])
                    c_new[d] = cn
                    ht = pt.tile([128, 128], F32, tag="ht")
                    nc.tensor.transpose(ht[:], h2[:], ident[:])
                    nc.vector.tensor_copy(sendA[:, d * 128:(d + 1) * 128], ht[:])
                    nc.sync.dma_start(out=agA_in[:, d * 128:(d + 1) * 128],
                                      in_=sendA[:, d * 128:(d + 1) * 128])
                # -- 2R) L1 gate partials: bias + w_hh1 (need h1T(t-1))
                g1 = []
                for d in range(2):
                    g = pg.tile([128, 512], F32, tag=f"g1{d}")
                    nc.tensor.matmul(g[:], (ones[:]), (b1_sb[d][:]),
                                     start=True, stop=False)
                    for k in range(8):
                        nc.tensor.matmul(g[:], (h_prev[2 + d][:, k, :]),
                                         (w1h_sb[d][:, k, :]),
                                         start=False, stop=False)
                    g1.append(g)
                # -- 6) AllGather A (h0 slices)
                agA_out = dp.tile([1024, 256], MMD, tag="agAo", addr_space="Shared")
                nc.gpsimd.collective_compute(
                    "AllGather", ALU.bypass, replica_groups=RG,
                    ins=[agA_in.opt()], outs=[agA_out.opt()],
                )
                h0T_new = []
                for d in range(2):
                    t_ = hp.tile([128, 8, 128], MMD, tag=f"h{d}")
                    nc.sync.dma_start(
                        out=t_[:],
                        in_=agA_out[:, d * 128:(d + 1) * 128].rearrange(
                            "(k p) b -> p k b", p=128),
                    )
                    h0T_new.append(t_)
                    h_new[d] = t_
                # -- 7) close L1 gates: w_ih1 over gathered h0 (k-major for overlap)
                for sd in range(2):
                    for k in range(8):
                        for d in range(2):
                            last = sd == 1 and k == 7 and d == 1
                            nc.tensor.matmul(
                                g1[d][:], (h0T_new[sd][:, k, :]),
                                (w1i_sb[d][:, sd * 8 + k, :]),
                                start=False, stop=last,
                            )
                # -- 8) L1 elementwise + transpose + lin partials
                sendBh = sp.tile([128, 256], MMD, tag="sendBh")
                agB_in = dp.tile([128, 320], MMD, tag="agBi")
                sendBl = sp.tile([128, 64], F32, tag="sendBl")
                for d in range(2):
                    a = gp.tile([128, 512], F32, tag=f"a{d}")
                    nc.scalar.activation(a[:, 0:384], g1[d][:, 0:384], AF.Sigmoid)
                    nc.scalar.activation(a[:, 384:512], g1[d][:, 384:512], AF.Tanh)
                    t1 = ewp.tile([128, 128], F32, tag="t1")
                    nc.vector.tensor_mul(t1[:], a[:, 128:256], c_prev[2 + d][:])
                    t2 = ewp.tile([128, 128], F32, tag="t2")
                    nc.vector.tensor_mul(t2[:], a[:, 0:128], a[:, 384:512])
                    cn = cp.tile([128, 128], F32, tag=f"c{2 + d}")
                    nc.vector.tensor_add(cn[:], t1[:], t2[:])
                    tc2 = ewp.tile([128, 128], F32, tag="tc2")
                    nc.scalar.activation(tc2[:], cn[:], AF.Tanh)
                    h2 = gp.tile([128, 128], F32, tag=f"h2_{2 + d}")
                    nc.vector.tensor_mul(h2[:], a[:, 256:384], tc2[:])
                    c_new[2 + d] = cn
                    ht = pt.tile([128, 128], F32, tag="ht")
                    nc.tensor.transpose(ht[:], h2[:], ident[:])
                    nc.vector.tensor_copy(sendBh[:, d * 128:(d + 1) * 128], ht[:])
                    nc.sync.dma_start(out=agB_in[:, d * 128:(d + 1) * 128],
                                      in_=sendBh[:, d * 128:(d + 1) * 128])
                lp = px.tile([128, 64], F32, tag="lp")
                for d in range(2):
                    nc.tensor.matmul(lp[:], (sendBh[:, d * 128:(d + 1) * 128]),
                                     (lin_sb[:, d, :]),
                                     start=(d == 0), stop=(d == 1))
                nc.vector.tensor_copy(sendBl[:], lp[:])
                # -- 9) AllGather B (h1 slices + logit partials)
                nc.sync.dma_start(out=agB_in[:, 256:320].bitcast(F32), in_=sendBl[:])
                agB_out = dp.tile([1024, 320], MMD, tag="agBo", addr_space="Shared")
                nc.gpsimd.collective_compute(
                    "AllGather", ALU.bypass, replica_groups=RG,
                    ins=[agB_in.opt()], outs=[agB_out.opt()],
                )
                LG = gp.tile([128, 8, 64], F32, tag="LG")
                nc.sync.dma_start(
                    out=LG[:],
                    in_=agB_out[:, 256:320].bitcast(F32).rearrange("(c p) v -> p c v", p=128),
                )
                for d in range(2):
                    t_ = hp.tile([128, 8, 128], MMD, tag=f"h{2 + d}")
                    nc.sync.dma_start(
                        out=t_[:],
                        in_=agB_out[:, d * 128:(d + 1) * 128].rearrange(
                            "(k p) b -> p k b", p=128),
                    )
                    h_new[2 + d] = t_
                # -- 10) logits sum + argmax + flag + masked store
                L = ap_.tile([128, 64], F32, tag="L")
                l4 = gp.tile([128, 4, 64], F32, tag="l4")
                nc.vector.tensor_add(l4[:], LG[:, 0:4, :], LG[:, 4:8, :])
                l2 = gp.tile([128, 2, 64], F32, tag="l2")
                nc.vector.tensor_add(l2[:], l4[:, 0:2, :], l4[:, 2:4, :])
                nc.vector.tensor_add(L[:], l2[:, 0, :], l2[:, 1, :])
                nc.vector.tensor_add(L[:], L[:], linb_sb[:])
                m = ap_.tile([128, 1], F32, tag="m")
                nc.vector.tensor_reduce(m[:], L[:], axis=mybir.AxisListType.X,
                                        op=ALU.max)
                ismax = ap_.tile([128, 64], F32, tag="ismax")
                nc.vector.tensor_scalar(ismax[:], L[:], m[:], None, op0=ALU.is_ge)
                cand = ap_.tile([128, 64], F32, tag="cand")
                nc.vector.tensor_mul(cand[:], ismax[:], iot_sb[:])
                nc.vector.tensor_scalar(cand[:], cand[:], 100.0, None, op0=ALU.add)
                idx = ap_.tile([128, 1], F32, tag="idx")
                nc.vector.tensor_reduce(idx[:], cand[:], axis=mybir.AxisListType.X,
                                        op=ALU.min)
                flagb = ap_.tile([128, 1], F32, tag="flagb")
                nc.vector.tensor_scalar(flagb[:], idx[:], 1.0, None, op0=ALU.is_equal)
                fnew = ap_.tile([128, 1], F32, tag="flag")
                nc.vector.tensor_max(fnew[:], flag_prev[:], flagb[:])
                tk = ap_.tile([128, 64], F32, tag="tk")
                nc.vector.tensor_mul(tk[:], L[:], nk_sb[:])
                tk2 = ap_.tile([128, 64], F32, tag="tk2")
                nc.vector.tensor_scalar(tk2[:], tk[:], fnew[:], None, op0=ALU.mult)
                if t % CHUNK == 0:
                    ybuf = yp.tile([128, CHUNK, 64], F32, tag="ybuf")
                nc.vector.tensor_sub(ybuf[:, t % CHUNK, :], L[:], tk2[:])
                if t % CHUNK == CHUNK - 1:
                    nc.sync.dma_start(out=y[:, t - CHUNK + 1:t + 1, :], in_=ybuf[:])
                # carry
                h_prev = h_new
                c_prev = c_new
                flag_prev = fnew
                idx_prev = idx
            if T % CHUNK != 0:
                nfin = T % CHUNK
                nc.sync.dma_start(out=y[:, T - nfin:T, :], in_=ybuf[:, 0:nfin, :])
            for cell in range(4):
                nc.sync.dma_start(out=hT_f[cell], in_=h_prev[cell][:])
                nc.sync.dma_start(out=c_f[cell], in_=c_prev[cell][:])
            nc.sync.dma_start(out=flag_f[:], in_=flag_prev[:])
            nc.sync.dma_start(out=idx_f[:], in_=idx_prev[:])
    nc.finalize()
    return nc


def prep_inputs(h0, c0, w_ih0, w_hh0, b0, w_ih1, w_hh1, b1, lin_w, lin_b):
    """Host-side packing: per-core sliced/transposed weight + state arrays."""
    h0 = np.asarray(h0, np.float32).reshape(2, 2, B, H)
    c0 = np.asarray(c0, np.float32).reshape(2, 2, B, H)
    w_ih0 = np.asarray(w_ih0, np.float32)
    w_hh0 = np.asarray(w_hh0, np.float32)
    b0 = np.asarray(b0, np.float32)
    w_ih1 = np.asarray(w_ih1, np.float32)
    w_hh1 = np.asarray(w_hh1, np.float32)
    b1 = np.asarray(b1, np.float32)
    lin_w = np.asarray(lin_w, np.float32)
    lin_b = np.asarray(lin_b, np.float32)

    iota = np.broadcast_to((np.arange(V) - 100.0).astype(np.float32), (128, V)).copy()
    linbb = np.broadcast_to(lin_b, (128, V)).copy()
    nk = np.ones((128, V), np.float32)
    nk[:, KEEP_IDX] = 0.0

    hT0 = np.zeros((4, 128, 8, B), np.float32)
    for l in range(2):
        for d in range(2):
            cell = l * 2 + d
            hT0[cell] = h0[l, d].T.reshape(8, 128, B).transpose(1, 0, 2)

    in_maps = []
    for c in range(NCORES):
        rows = np.concatenate([np.arange(gb + c * 128, gb + c * 128 + 128)
                               for gb in GBASE])

        def packT(w, kt):
            # w: (4H, K*128) -> select rows -> [p, k, n]
            sel = w[rows, :]  # (512, kt*128)
            return np.ascontiguousarray(
                sel.reshape(512, kt, 128).transpose(2, 1, 0))

        w0T = np.stack([packT(w_hh0[d], 8) for d in range(2)])
        w1iT = np.stack([packT(w_ih1[d], 16) for d in range(2)])
        w1hT = np.stack([packT(w_hh1[d], 8) for d in range(2)])
        w0aug = np.stack([np.stack([w_ih0[d][rows, 0], b0[d][rows]])
                          for d in range(2)])
        b1row = np.stack([b1[d][rows][None, :] for d in range(2)])
        linTc = np.stack(
            [lin_w[:, c * 128:(c + 1) * 128].T,
             lin_w[:, H + c * 128:H + (c + 1) * 128].T], axis=1)
        c0slice = np.zeros((4, 128, 128), np.float32)
        for l in range(2):
            for d in range(2):
                c0slice[l * 2 + d] = c0[l, d][:, c * 128:(c + 1) * 128]
        in_maps.append({
            "w0T": tf32_round(np.ascontiguousarray(w0T)),
            "w0aug": tf32_round(np.ascontiguousarray(w0aug)),
            "w1iT": tf32_round(np.ascontiguousarray(w1iT)),
            "w1hT": tf32_round(np.ascontiguousarray(w1hT)),
            "b1row": tf32_round(np.ascontiguousarray(b1row)),
            "linTc": tf32_round(np.ascontiguousarray(linTc)),
            "linb": linbb,
            "iotam": iota,
            "notkeep": nk,
            "hT0": tf32_round(hT0),
            "c0s": np.ascontiguousarray(c0slice),
            "onesrow": np.ones((1, 128), np.float32),
            "x0row": np.full((1, 128), MASK_IDX, np.float32),
            "flag0": np.zeros((128, 1), np.float32),
        })
    return in_maps


_NC_CACHE = {}


def _get_nc(T):
    if T not in _NC_CACHE:
        _NC_CACHE[T] = build(T)
    return _NC_CACHE[T]


T_LAUNCH = 256


def kernel(h0, c0, w_ih0, w_hh0, b0, w_ih1, w_hh1, b1, lin_w, lin_b,
           decoder_output_length, batch_size, _want_results=False):
    T = int(decoder_output_length)
    assert int(batch_size) == B
    in_maps = prep_inputs(h0, c0, w_ih0, w_hh0, b0, w_ih1, w_hh1, b1,
                          lin_w, lin_b)
    chunks = []
    t_done = 0
    res = None
    while t_done < T:
        t_this = min(T_LAUNCH, T - t_done)
        nc = _get_nc(t_this)
        res = bass_utils.run_bass_kernel_spmd(nc, in_maps,
                                              core_ids=list(range(NCORES)))
        chunks.append(res.results[0]["y"])
        t_done += t_this
        if t_done < T:
            idxs = res.results[0]["idx_f"]  # (128,1) float indices
            xrow = np.ascontiguousarray(idxs.reshape(1, 128))
            for c in range(NCORES):
                rc = res.results[c]
                in_maps[c] = dict(in_maps[c])
                in_maps[c]["hT0"] = rc["hT_f"]
                in_maps[c]["c0s"] = rc["c_f"]
                in_maps[c]["flag0"] = rc["flag_f"]
                in_maps[c]["x0row"] = xrow
    out = np.concatenate(chunks, axis=1) if len(chunks) > 1 else chunks[0]
    if _want_results:
        return out, res
    return out



# revision 55
# speedup vs baseline: 1.3310x; 1.3310x over previous
"""Trainium2 Bass kernel for nn_Decoder (2-layer bidirectional LSTM decoder,
autoregressive argmax feedback, T=512 steps, B=128, H=1024, V=64).

Strategy: 8-way tensor parallel over the 4H gate dimension. Each core holds a
512-wide slice of every gate projection (re-ordered [i,f,o,g]), keeps the LSTM
recurrence state resident, and exchanges the 128-col h-slices it owns via two
AllGathers per step. Matmuls run as fp32r with the transposed h-state as the
stationary operand and the weight slice as the 512-wide moving operand. Logits
are computed as per-core partials, summed after the second AllGather, and the
argmax (via DVE max_index) feeds the next step on-device.

v2 restructure vs the original baseline:
 - single batched DMA per AllGather input; readbacks split per direction so
   L1 input matmuls start as soon as the first half lands
 - argmax via max/max_index (3 DVE ops) instead of is_ge/iota/min chain
 - y-masking DVE ops emitted after the L0 elementwise chain so they do not
   block the x-feedback path on the in-order vector engine
 - logit partial reduction as LG[128,8,64] + pairwise adds
 - PE program order arranged so gate partials for the next cell run inside
   the AllGather windows
"""

import os
import sys

import numpy as np

sys.path.insert(0, "/opt/trn_rl_repo")

import concourse.bass as bass  # noqa: E402
import concourse.mybir as mybir  # noqa: E402
import concourse.tile as tile  # noqa: E402
from concourse import bacc  # noqa: E402
from concourse import bass_utils  # noqa: E402
from concourse.masks import make_identity  # noqa: E402

H = 1024
V = 64
B = 128
NCORES = 8
MASK_IDX = 4.0
KEEP_IDX = 3
BLANK_IDX = 1.0
T_STEPS = int(os.environ.get("DEC_T", "512"))
CHUNK = int(os.environ.get("DEC_CHUNK", "16"))
MM_DT = mybir.dt.float32r if os.environ.get("DEC_MMDT", "fp32r") == "fp32r" else mybir.dt.float32
F32 = mybir.dt.float32
U32 = mybir.dt.uint32
MMD = MM_DT
AF = mybir.ActivationFunctionType
ALU = mybir.AluOpType
# number of DMAs the post-AllGather h-state readback is split into (per dir
# halves further split to stagger matmul sem releases)
RB_SPLIT = int(os.environ.get("DEC_RB", "16"))

# gate blocks packed per-core as [i, f, o, g] (torch order in rows is i,f,g,o)
GBASE = [0, H, 3 * H, 2 * H]


def tf32_round(x):
    if MM_DT == F32:
        return np.asarray(x, np.float32)
    xi = np.asarray(x, np.float32).view(np.uint32)
    xi = (xi + np.uint32(1 << 12)) & np.uint32(0xFFFFE000)
    return xi.view(np.float32)


def build(T=T_STEPS):
    nc = bacc.Bacc("TRN2", num_devices=NCORES)
    RG = [list(range(NCORES))]

    din = dict(kind="ExternalInput")
    w0c = nc.dram_tensor("w0c", [128, 8, 1024], MMD, **din)      # whh0^T  d0|d1
    w1ic = nc.dram_tensor("w1ic", [128, 16, 1024], MMD, **din)   # wih1^T  d0|d1
    w1hc = nc.dram_tensor("w1hc", [128, 8, 1024], MMD, **din)    # whh1^T  d0|d1
    w0xrep = nc.dram_tensor("w0xrep", [1, 1024], MMD, **din)     # x-weights row
    b0c = nc.dram_tensor("b0c", [1, 1024], MMD, **din)
    b1c = nc.dram_tensor("b1c", [1, 1024], MMD, **din)
    linTc = nc.dram_tensor("linTc", [128, 2, 64], MMD, **din)
    linb8 = nc.dram_tensor("linb8", [1, 64], MMD, **din)
    notkeep = nc.dram_tensor("notkeep", [128, 64], F32, **din)
    hT0_0 = nc.dram_tensor("hT0_0", [128, 16, 128], MMD, **din)  # h0T chunks d0|d1
    hT0_1 = nc.dram_tensor("hT0_1", [128, 16, 128], MMD, **din)  # h1T chunks d0|d1
    c0c = nc.dram_tensor("c0c", [2, 128, 256], F32, **din)       # c state  d0|d1
    onesrow = nc.dram_tensor("onesrow", [1, 128], MMD, **din)
    idx0 = nc.dram_tensor("idx0", [128, 1], F32, **din)
    flag0 = nc.dram_tensor("flag0", [128, 1], F32, **din)
    hTf_0 = nc.dram_tensor("hTf_0", [128, 16, 128], MMD, kind="ExternalOutput")
    hTf_1 = nc.dram_tensor("hTf_1", [128, 16, 128], MMD, kind="ExternalOutput")
    c_f = nc.dram_tensor("c_f", [2, 128, 256], F32, kind="ExternalOutput")
    flag_f = nc.dram_tensor("flag_f", [128, 1], F32, kind="ExternalOutput")
    idx_f = nc.dram_tensor("idx_f", [128, 1], F32, kind="ExternalOutput")
    y = nc.dram_tensor("y", [B, T, V], F32, kind="ExternalOutput")

    with tile.TileContext(nc) as tc:
        import contextlib

        ctx = contextlib.ExitStack()
        with ctx:
            wp = ctx.enter_context(tc.tile_pool(name="weights", bufs=1))
            hp = ctx.enter_context(tc.tile_pool(name="hstate", bufs=1))
            cp = ctx.enter_context(tc.tile_pool(name="cstate", bufs=2))
            gp = ctx.enter_context(tc.tile_pool(name="gact", bufs=1))
            lgp = ctx.enter_context(tc.tile_pool(name="lgpool", bufs=2))
            sp = ctx.enter_context(tc.tile_pool(name="send", bufs=2))
            ap_ = ctx.enter_context(tc.tile_pool(name="amax", bufs=2))
            yp = ctx.enter_context(tc.tile_pool(name="ybuf", bufs=2))
            pg = ctx.enter_context(tc.tile_pool(name="pgates", bufs=1, space="PSUM"))
            pt = ctx.enter_context(tc.tile_pool(name="ptrans", bufs=1, space="PSUM"))
            px = ctx.enter_context(tc.tile_pool(name="pmisc", bufs=1, space="PSUM"))
            dp = ctx.enter_context(tc.tile_pool(name="dram", bufs=2, space="DRAM"))

            # ---- load weights + constants (once) ----
            w0_sb = wp.tile([128, 8, 1024], MMD, tag="w0")
            nc.sync.dma_start(out=w0_sb[:], in_=w0c[:])
            w1i_sb = wp.tile([128, 16, 1024], MMD, tag="w1i")
            nc.sync.dma_start(out=w1i_sb[:], in_=w1ic[:])
            w1h_sb = wp.tile([128, 8, 1024], MMD, tag="w1h")
            nc.sync.dma_start(out=w1h_sb[:], in_=w1hc[:])
            w0x_sb = wp.tile([1, 1024], MMD, tag="w0x")
            nc.sync.dma_start(out=w0x_sb[:], in_=w0xrep[:])
            b0_sb = wp.tile([1, 1024], MMD, tag="b0")
            nc.sync.dma_start(out=b0_sb[:], in_=b0c[:])
            b1_sb = wp.tile([1, 1024], MMD, tag="b1")
            nc.sync.dma_start(out=b1_sb[:], in_=b1c[:])
            lin_sb = wp.tile([128, 2, 64], MMD, tag="lin")
            nc.sync.dma_start(out=lin_sb[:], in_=linTc[:])
            linb8_sb = wp.tile([1, 64], MMD, tag="linb8")
            nc.sync.dma_start(out=linb8_sb[:], in_=linb8[:])
            nk_sb = wp.tile([128, 64], F32, tag="nk")
            nc.sync.dma_start(out=nk_sb[:], in_=notkeep[:])
            ident = wp.tile([128, 128], F32, tag="ident")
            make_identity(nc, ident[:])
            ones = wp.tile([1, 128], MMD, tag="ones")
            nc.sync.dma_start(out=ones[:], in_=onesrow[:])

            # ---- initial state ----
            h0T = hp.tile([128, 16, 128], MMD, tag="h0T")
            nc.sync.dma_start(out=h0T[:], in_=hT0_0[:])
            h1T = hp.tile([128, 16, 128], MMD, tag="h1T")
            nc.sync.dma_start(out=h1T[:], in_=hT0_1[:])
            cc0 = cp.tile([128, 256], F32, tag="c0")
            nc.sync.dma_start(out=cc0[:], in_=c0c[0])
            cc1 = cp.tile([128, 256], F32, tag="c1")
            nc.sync.dma_start(out=cc1[:], in_=c0c[1])
            flag_prev = ap_.tile([128, 1], F32, tag="flag")
            nc.sync.dma_start(out=flag_prev[:], in_=flag0[:])
            idxf = ap_.tile([128, 1], F32, tag="idx")
            nc.sync.dma_start(out=idxf[:], in_=idx0[:])

            def g_partials(tag, bias_sb, w_sb, hT_tile, ones_t=None,
                           close_group=False):
                """bias + w_hh partial accumulation for one layer (both dirs).
                Returns the two PSUM gate tiles [128, 512] (d0, d1)."""
                if ones_t is None:
                    ones_t = ones
                g = [pg.tile([128, 512], F32, tag=f"{tag}{d}", name=f"{tag}{d}")
                     for d in range(2)]
                for d in range(2):
                    nc.tensor.matmul(g[d][:], ones_t[:],
                                     bias_sb[:, d * 512:(d + 1) * 512],
                                     start=True, stop=False)
                for k in range(8):
                    for d in range(2):
                        nc.tensor.matmul(g[d][:], hT_tile[:, d * 8 + k, :],
                                         w_sb[:, k, d * 512:(d + 1) * 512],
                                         start=False,
                                         stop=(close_group and k == 7))
                return g

            def lstm_ew(g, cc_prev, cc_tag, sendT, per_dir=None):
                """Elementwise LSTM cell for both dirs from PSUM gates g[d];
                transposes each dir's h2 half into sendT as soon as it is
                ready. Returns (cc_new [128,256], h2 [128,256])."""
                cc_new = cp.tile([128, 256], F32, tag=cc_tag)
                h2 = gp.tile([128, 256], F32, tag=f"h2_{cc_tag}")
                for d in range(2):
                    a = gp.tile([128, 512], F32, tag=f"a{cc_tag}{d}")
                    nc.scalar.activation(a[:, 0:256], g[d][:, 0:256], AF.Sigmoid)
                    nc.scalar.activation(a[:, 384:512], g[d][:, 384:512], AF.Tanh)
                    nc.scalar.activation(a[:, 256:384], g[d][:, 256:384], AF.Sigmoid)
                    t1 = gp.tile([128, 128], F32, tag=f"t1{d}")
                    nc.vector.tensor_mul(t1[:], a[:, 128:256],
                                         cc_prev[:, d * 128:(d + 1) * 128])
                    t2 = gp.tile([128, 128], F32, tag=f"t2{d}")
                    nc.vector.tensor_mul(t2[:], a[:, 0:128], a[:, 384:512])
                    nc.vector.tensor_add(cc_new[:, d * 128:(d + 1) * 128],
                                         t1[:], t2[:])
                    tc2 = gp.tile([128, 128], F32, tag=f"tc2{d}")
                    nc.scalar.activation(tc2[:], cc_new[:, d * 128:(d + 1) * 128],
                                         AF.Tanh)
                    nc.vector.tensor_mul(h2[:, d * 128:(d + 1) * 128],
                                         a[:, 256:384], tc2[:])
                    nc.tensor.transpose(sendT[:, d * 128:(d + 1) * 128],
                                        h2[:, d * 128:(d + 1) * 128], ident[:])
                    if per_dir is not None:
                        per_dir(d)
                return cc_new, h2

            # prologue: accumulate L0 gate partials for step 0
            g0 = g_partials("g0", b0_sb, w0_sb, h0T)

            LG = None
            Lsum = None
            ybuf = None

            for t in range(T):
                # ---- argmax of logits(t-1) -> x(t) ----
                if t > 0:
                    l4 = ap_.tile([128, 4, 64], F32, tag="l4")
                    nc.vector.tensor_add(l4[:], LG[:, 0:4, :], LG[:, 4:8, :])
                    l2 = ap_.tile([128, 2, 64], F32, tag="l2")
                    nc.vector.tensor_add(l2[:], l4[:, 0:2, :], l4[:, 2:4, :])
                    Lsum = ap_.tile([128, 64], F32, tag="L")
                    nc.vector.tensor_add(Lsum[:], l2[:, 0, :], l2[:, 1, :])
                    m8 = ap_.tile([128, 8], F32, tag="m8")
                    nc.vector.max(m8[:], Lsum[:])
                    mi = ap_.tile([128, 8], U32, tag="mi")
                    nc.vector.max_index(mi[:], m8[:], Lsum[:])
                    idxf = ap_.tile([128, 1], F32, tag="idx")
                    nc.vector.tensor_copy(idxf[:], mi[:, 0:1])
                # ---- close L0 gates with x contribution ----
                x_ps = px.tile([1, 128], F32, tag="xps")
                nc.tensor.transpose(x_ps[:], idxf[:], ident[:])
                x_row = ap_.tile([1, 128], MMD, tag="xrow")
                nc.vector.tensor_copy(x_row[:], x_ps[:])
                for d in range(2):
                    nc.tensor.matmul(g0[d][:], x_row[:],
                                     w0x_sb[:, d * 512:(d + 1) * 512],
                                     start=False, stop=True)
                # ---- L0 elementwise + transpose own h0 slices ----
                sendT0 = pt.tile([128, 256], F32, tag="sendT0")
                cc0, h2_0 = lstm_ew(g0, cc0, "c0", sendT0)
                sendA = sp.tile([128, 256], MMD, tag="sendA")
                nc.vector.tensor_copy(sendA[:], sendT0[:])
                agA_in = dp.tile([128, 256], MMD, tag="agAi")
                nc.sync.dma_start(out=agA_in[:], in_=sendA[:])
                # pacer: releases the g1 partial group only once the L0 send
                # path has drained, keeping those matmuls out of its way
                onesg1 = ap_.tile([1, 128], MMD, tag="onesg1")
                nc.vector.tensor_scalar(onesg1[:], sendA[0:1, 128:256], 0.0, 1.0,
                                        op0=ALU.mult, op1=ALU.add)
                # ---- flag/mask/y for step t-1 (off critical path) ----
                if t > 0:
                    flagb = ap_.tile([128, 1], F32, tag="flagb")
                    nc.vector.tensor_scalar(flagb[:], idxf[:], BLANK_IDX, None,
                                            op0=ALU.is_equal)
                    fnew = ap_.tile([128, 1], F32, tag="flag")
                    nc.vector.tensor_max(fnew[:], flag_prev[:], flagb[:])
                    flag_prev = fnew
                    tk2 = ap_.tile([128, 64], F32, tag="tk2")
                    nc.vector.scalar_tensor_tensor(tk2[:], nk_sb[:], fnew[:],
                                                   Lsum[:], op0=ALU.mult,
                                                   op1=ALU.mult)
                    s = t - 1
                    if s % CHUNK == 0:
                        ybuf = yp.tile([128, CHUNK, 64], F32, tag="ybuf")
                    nc.vector.tensor_sub(ybuf[:, s % CHUNK, :], Lsum[:], tk2[:])
                    if s % CHUNK == CHUNK - 1:
                        nc.sync.dma_start(out=y[:, s - CHUNK + 1:s + 1, :],
                                          in_=ybuf[:])
                # ---- AllGather A (h0 slices) ----
                agA_out = dp.tile([1024, 256], MMD, tag="agAo", addr_space="Shared")
                nc.gpsimd.collective_compute(
                    "AllGather", ALU.bypass, replica_groups=RG,
                    ins=[agA_in.opt()], outs=[agA_out.opt()],
                )
                # ---- L1 gate partials (run during AllGather A) ----
                g1 = g_partials("g1", b1_sb, w1h_sb, h1T, ones_t=onesg1)
                # ---- readback gathered h0T (split so matmul releases stagger) ----
                h0T = hp.tile([128, 16, 128], MMD, tag="h0T")
                rb_bounds = []
                lo = 0
                for r in range(RB_SPLIT):
                    hi = ((r + 1) * 16) // RB_SPLIT
                    rb_bounds.append((lo, hi))
                    lo = hi
                # interleave HWDGE (sync) and SWDGE (gpsimd) issue queues so
                # chunk arrival matches consumption order: merge of a ~650ns
                # HWDGE stream and a ~1040ns SWDGE stream
                rb_eng = [0, 1, 0, 0, 1, 0, 1, 0, 0, 1, 0, 0, 1, 0, 1, 0]
                for ri, (lo, hi) in enumerate(rb_bounds):
                    # chunk kk = d*8+k lives at agA_out[(kk%8)*128 + p, (kk//8)*128 + b]
                    eng = nc.sync if rb_eng[ri % 16] == 0 else nc.gpsimd
                    for dd in (0, 1):
                        l2_, h2_ = max(lo, dd * 8), min(hi, (dd + 1) * 8)
                        if l2_ >= h2_:
                            continue
                        eng.dma_start(
                            out=h0T[:, l2_:h2_, :],
                            in_=agA_out[(l2_ - dd * 8) * 128:(h2_ - dd * 8) * 128,
                                        dd * 128:(dd + 1) * 128].rearrange(
                                "(k p) b -> p k b", p=128),
                        )
                # ---- close L1 gates: w_ih1 over gathered h0 ----
                for kk in range(16):
                    for d in range(2):
                        nc.tensor.matmul(
                            g1[d][:], h0T[:, kk, :],
                            w1i_sb[:, kk, d * 512:(d + 1) * 512],
                            start=False, stop=(kk == 15),
                        )
                # ---- L1 elementwise + transpose own h1 slices ----
                sendT1 = pt.tile([128, 256], F32, tag="sendT1")
                cc1, h2_1 = lstm_ew(g1, cc1, "c1", sendT1)
                sendB = sp.tile([128, 256], MMD, tag="sendB")
                nc.vector.tensor_copy(sendB[:], sendT1[:])
                agB_in = dp.tile([128, 320], MMD, tag="agBi")
                nc.sync.dma_start(out=agB_in[:, 0:256], in_=sendB[:])
                lp = px.tile([128, 64], F32, tag="lp")
                nc.tensor.matmul(lp[:], ones[:], linb8_sb[:],
                                 start=True, stop=False)
                for d in range(2):
                    nc.tensor.matmul(lp[:], sendB[:, d * 128:(d + 1) * 128],
                                     lin_sb[:, d, :],
                                     start=False, stop=(d == 1))
                lpc = sp.tile([128, 64], F32, tag="lpc")
                nc.vector.tensor_copy(lpc[:], lp[:])
                nc.sync.dma_start(out=agB_in[:, 256:320].bitcast(F32), in_=lpc[:])
                # pacer: releases the next-step g0 partial group only after the
                # L1 send path (incl. logit partials) has drained
                p1 = ap_.tile([1, 1], F32, tag="p1")
                nc.vector.tensor_scalar(p1[:], lpc[0:1, 0:1], 0.0, 1.0,
                                        op0=ALU.mult, op1=ALU.add)
                onesg0 = ap_.tile([1, 128], MMD, tag="onesg0")
                nc.vector.tensor_scalar(onesg0[:], ones[:], p1[:], None,
                                        op0=ALU.mult)
                # ---- AllGather B (h1 slices + logit partials) ----
                agB_out = dp.tile([1024, 320], MMD, tag="agBo", addr_space="Shared")
                nc.gpsimd.collective_compute(
                    "AllGather", ALU.bypass, replica_groups=RG,
                    ins=[agB_in.opt()], outs=[agB_out.opt()],
                )
                # ---- readbacks: logit partials FIRST (argmax path), then h1T ----
                LG = lgp.tile([128, 8, 64], F32, tag="LG")
                nc.sync.dma_start(
                    out=LG[:],
                    in_=agB_out[:, 256:320].bitcast(F32).rearrange(
                        "(c p) v -> p c v", p=128),
                )
                h1T = hp.tile([128, 16, 128], MMD, tag="h1T")
                for dd in (0, 1):
                    nc.gpsimd.dma_start(
                        out=h1T[:, dd * 8:(dd + 1) * 8, :],
                        in_=agB_out[:, dd * 128:(dd + 1) * 128].rearrange(
                            "(k p) b -> p k b", p=128),
                    )
                # ---- L0 gate partials for step t+1 (run during AllGather B) ----
                g0 = g_partials("g0", b0_sb, w0_sb, h0T, ones_t=onesg0)

            # ---- epilogue: argmax/flag/y for final step ----
            l4 = ap_.tile([128, 4, 64], F32, tag="l4")
            nc.vector.tensor_add(l4[:], LG[:, 0:4, :], LG[:, 4:8, :])
            l2 = ap_.tile([128, 2, 64], F32, tag="l2")
            nc.vector.tensor_add(l2[:], l4[:, 0:2, :], l4[:, 2:4, :])
            Lsum = ap_.tile([128, 64], F32, tag="L")
            nc.vector.tensor_add(Lsum[:], l2[:, 0, :], l2[:, 1, :])
            m8 = ap_.tile([128, 8], F32, tag="m8")
            nc.vector.max(m8[:], Lsum[:])
            mi = ap_.tile([128, 8], U32, tag="mi")
            nc.vector.max_index(mi[:], m8[:], Lsum[:])
            idxf = ap_.tile([128, 1], F32, tag="idx")
            nc.vector.tensor_copy(idxf[:], mi[:, 0:1])
            flagb = ap_.tile([128, 1], F32, tag="flagb")
            nc.vector.tensor_scalar(flagb[:], idxf[:], BLANK_IDX, None,
                                    op0=ALU.is_equal)
            fnew = ap_.tile([128, 1], F32, tag="flag")
            nc.vector.tensor_max(fnew[:], flag_prev[:], flagb[:])
            tk2 = ap_.tile([128, 64], F32, tag="tk2")
            nc.vector.scalar_tensor_tensor(tk2[:], nk_sb[:], fnew[:], Lsum[:],
                                           op0=ALU.mult, op1=ALU.mult)
            s = T - 1
            nc.vector.tensor_sub(ybuf[:, s % CHUNK, :], Lsum[:], tk2[:])
            nfin = (s % CHUNK) + 1
            nc.sync.dma_start(out=y[:, T - nfin:T, :], in_=ybuf[:, 0:nfin, :])
            # ---- final state stores ----
            nc.sync.dma_start(out=hTf_0[:], in_=h0T[:])
            nc.sync.dma_start(out=hTf_1[:], in_=h1T[:])
            nc.sync.dma_start(out=c_f[0], in_=cc0[:])
            nc.sync.dma_start(out=c_f[1], in_=cc1[:])
            nc.sync.dma_start(out=flag_f[:], in_=fnew[:])
            nc.sync.dma_start(out=idx_f[:], in_=idxf[:])
    nc.finalize()
    return nc


def prep_inputs(h0, c0, w_ih0, w_hh0, b0, w_ih1, w_hh1, b1, lin_w, lin_b):
    """Host-side packing: per-core sliced/transposed weight + state arrays."""
    h0 = np.asarray(h0, np.float32).reshape(2, 2, B, H)
    c0 = np.asarray(c0, np.float32).reshape(2, 2, B, H)
    w_ih0 = np.asarray(w_ih0, np.float32)
    w_hh0 = np.asarray(w_hh0, np.float32)
    b0 = np.asarray(b0, np.float32)
    w_ih1 = np.asarray(w_ih1, np.float32)
    w_hh1 = np.asarray(w_hh1, np.float32)
    b1 = np.asarray(b1, np.float32)
    lin_w = np.asarray(lin_w, np.float32)
    lin_b = np.asarray(lin_b, np.float32)

    linbb = np.broadcast_to(lin_b, (128, V)).copy()
    nk = np.ones((128, V), np.float32)
    nk[:, KEEP_IDX] = 0.0

    # initial transposed h state: [128, 16, 128], chunk kk = d*8+k
    def hT_init(l):
        out = np.zeros((128, 16, B), np.float32)
        for d in range(2):
            hT = h0[l, d].T.reshape(8, 128, B)          # [k, p, b]
            out[:, d * 8:(d + 1) * 8, :] = hT.transpose(1, 0, 2)
        return out

    hT0_0 = hT_init(0)
    hT0_1 = hT_init(1)

    in_maps = []
    for c in range(NCORES):
        rows = np.concatenate([np.arange(gb + c * 128, gb + c * 128 + 128)
                               for gb in GBASE])

        def packT(w, kt):
            # w: (4H, K*128) -> select rows -> [p, k, n]
            sel = w[rows, :]  # (512, kt*128)
            return np.ascontiguousarray(
                sel.reshape(512, kt, 128).transpose(2, 1, 0))

        w0c = np.concatenate([packT(w_hh0[d], 8) for d in range(2)], axis=2)
        w1ic = np.concatenate([packT(w_ih1[d], 16) for d in range(2)], axis=2)
        w1hc = np.concatenate([packT(w_hh1[d], 8) for d in range(2)], axis=2)
        w0xr = np.concatenate([w_ih0[0][rows, 0],
                               w_ih0[1][rows, 0]])[None, :]
        b0row = np.concatenate([b0[0][rows], b0[1][rows]])[None, :]
        b1row = np.concatenate([b1[0][rows], b1[1][rows]])[None, :]
        linTc = np.stack(
            [lin_w[:, c * 128:(c + 1) * 128].T,
             lin_w[:, H + c * 128:H + (c + 1) * 128].T], axis=1)
        c0slice = np.zeros((2, 128, 256), np.float32)
        for l in range(2):
            for d in range(2):
                c0slice[l, :, d * 128:(d + 1) * 128] = \
                    c0[l, d][:, c * 128:(c + 1) * 128]
        in_maps.append({
            "w0c": tf32_round(np.ascontiguousarray(w0c)),
            "w1ic": tf32_round(np.ascontiguousarray(w1ic)),
            "w1hc": tf32_round(np.ascontiguousarray(w1hc)),
            "w0xrep": tf32_round(np.ascontiguousarray(w0xr)),
            "b0c": tf32_round(b0row),
            "b1c": tf32_round(b1row),
            "linTc": tf32_round(np.ascontiguousarray(linTc)),
            "linb8": (lin_b / 8.0).reshape(1, V).astype(np.float32),
            "notkeep": nk,
            "hT0_0": tf32_round(hT0_0),
            "hT0_1": tf32_round(hT0_1),
            "c0c": np.ascontiguousarray(c0slice),
            "onesrow": np.ones((1, 128), np.float32),
            "idx0": np.full((128, 1), MASK_IDX, np.float32),
            "flag0": np.zeros((128, 1), np.float32),
        })
    return in_maps


_NC_CACHE = {}


def _get_nc(T):
    if T not in _NC_CACHE:
        _NC_CACHE[T] = build(T)
    return _NC_CACHE[T]


T_LAUNCH = 256


def kernel(h0, c0, w_ih0, w_hh0, b0, w_ih1, w_hh1, b1, lin_w, lin_b,
           decoder_output_length, batch_size, _want_results=False):
    T = int(decoder_output_length)
    assert int(batch_size) == B
    in_maps = prep_inputs(h0, c0, w_ih0, w_hh0, b0, w_ih1, w_hh1, b1,
                          lin_w, lin_b)
    chunks = []
    t_done = 0
    res = None
    while t_done < T:
        t_this = min(T_LAUNCH, T - t_done)
        nc = _get_nc(t_this)
        res = bass_utils.run_bass_kernel_spmd(nc, in_maps,
                                              core_ids=list(range(NCORES)))
        chunks.append(res.results[0]["y"])
        t_done += t_this
        if t_done < T:
            idxs = res.results[0]["idx_f"]  # (128,1) float indices
            for c in range(NCORES):
                rc = res.results[c]
                in_maps[c] = dict(in_maps[c])
                in_maps[c]["hT0_0"] = rc["hTf_0"]
                in_maps[c]["hT0_1"] = rc["hTf_1"]
                in_maps[c]["c0c"] = rc["c_f"]
                in_maps[c]["flag0"] = rc["flag_f"]
                in_maps[c]["idx0"] = np.ascontiguousarray(idxs)
    out = np.concatenate(chunks, axis=1) if len(chunks) > 1 else chunks[0]
    if _want_results:
        return out, res
    return out


# revision 65
# speedup vs baseline: 1.5719x; 1.1810x over previous
"""Trainium2 Bass kernel for nn_Decoder (2-layer bidirectional LSTM decoder,
autoregressive argmax feedback, T=512 steps, B=128, H=1024, V=64).

Strategy: 8-way tensor parallel over the 4H gate dimension. Each core holds a
512-wide slice of every gate projection (re-ordered [i,f,o,g]), keeps the LSTM
recurrence state resident, and exchanges the 128-col h-slices it owns via two
AllGathers per step. Matmuls run as fp32r with the transposed h-state as the
stationary operand and the weight slice as the 512-wide moving operand. Logits
are computed as per-core partials, summed after the second AllGather, and the
argmax (via DVE max_index) feeds the next step on-device.

v2 restructure vs the original baseline:
 - single batched DMA per AllGather input; readbacks split per direction so
   L1 input matmuls start as soon as the first half lands
 - argmax via max/max_index (3 DVE ops) instead of is_ge/iota/min chain
 - y-masking DVE ops emitted after the L0 elementwise chain so they do not
   block the x-feedback path on the in-order vector engine
 - logit partial reduction as LG[128,8,64] + pairwise adds
 - PE program order arranged so gate partials for the next cell run inside
   the AllGather windows
"""

import os
import sys

import numpy as np

sys.path.insert(0, "/opt/trn_rl_repo")

import concourse.bass as bass  # noqa: E402
import concourse.mybir as mybir  # noqa: E402
import concourse.tile as tile  # noqa: E402
from concourse import bacc  # noqa: E402
from concourse import bass_utils  # noqa: E402
from concourse.masks import make_identity  # noqa: E402

H = 1024
V = 64
B = 128
NCORES = 8
MASK_IDX = 4.0
KEEP_IDX = 3
BLANK_IDX = 1.0
T_STEPS = int(os.environ.get("DEC_T", "512"))
CHUNK = int(os.environ.get("DEC_CHUNK", "16"))
MM_DT = mybir.dt.float32r if os.environ.get("DEC_MMDT", "fp32r") == "fp32r" else mybir.dt.float32
F32 = mybir.dt.float32
U32 = mybir.dt.uint32
MMD = MM_DT
AF = mybir.ActivationFunctionType
ALU = mybir.AluOpType
# number of DMAs the post-AllGather h-state readback is split into (per dir
# halves further split to stagger matmul sem releases)
RB_SPLIT = int(os.environ.get("DEC_RB", "16"))
# chunk-group sizes for the post-AllGather h-state readback DMAs
RB_GROUPS = [int(x) for x in os.environ.get(
    "DEC_RBS", ",".join(["1"] * RB_SPLIT)).split(",")]

# gate blocks packed per-core as [i, f, o, g] (torch order in rows is i,f,g,o)
GBASE = [0, H, 3 * H, 2 * H]


def tf32_round(x):
    if MM_DT == F32:
        return np.asarray(x, np.float32)
    xi = np.asarray(x, np.float32).view(np.uint32)
    xi = (xi + np.uint32(1 << 12)) & np.uint32(0xFFFFE000)
    return xi.view(np.float32)


def build(T=T_STEPS):
    nc = bacc.Bacc("TRN2", num_devices=NCORES)
    RG = [list(range(NCORES))]

    din = dict(kind="ExternalInput")
    w0c = nc.dram_tensor("w0c", [128, 8, 1024], MMD, **din)      # whh0^T  d0|d1
    w1ic = nc.dram_tensor("w1ic", [128, 16, 1024], MMD, **din)   # wih1^T  d0|d1
    w1hc = nc.dram_tensor("w1hc", [128, 8, 1024], MMD, **din)    # whh1^T  d0|d1
    w0xrep = nc.dram_tensor("w0xrep", [1, 1024], MMD, **din)     # x-weights row
    b0c = nc.dram_tensor("b0c", [1, 1024], MMD, **din)
    b1c = nc.dram_tensor("b1c", [1, 1024], MMD, **din)
    linTc = nc.dram_tensor("linTc", [128, 2, 64], MMD, **din)
    linb8 = nc.dram_tensor("linb8", [1, 64], MMD, **din)
    notkeep = nc.dram_tensor("notkeep", [128, 64], F32, **din)
    hT0_0 = nc.dram_tensor("hT0_0", [128, 16, 128], MMD, **din)  # h0T chunks d0|d1
    hT0_1 = nc.dram_tensor("hT0_1", [128, 16, 128], MMD, **din)  # h1T chunks d0|d1
    c0c = nc.dram_tensor("c0c", [2, 128, 256], F32, **din)       # c state  d0|d1
    onesrow = nc.dram_tensor("onesrow", [1, 128], MMD, **din)
    idx0 = nc.dram_tensor("idx0", [128, 1], F32, **din)
    flag0 = nc.dram_tensor("flag0", [128, 1], F32, **din)
    hTf_0 = nc.dram_tensor("hTf_0", [128, 16, 128], MMD, kind="ExternalOutput")
    hTf_1 = nc.dram_tensor("hTf_1", [128, 16, 128], MMD, kind="ExternalOutput")
    c_f = nc.dram_tensor("c_f", [2, 128, 256], F32, kind="ExternalOutput")
    flag_f = nc.dram_tensor("flag_f", [128, 1], F32, kind="ExternalOutput")
    idx_f = nc.dram_tensor("idx_f", [128, 1], F32, kind="ExternalOutput")
    y = nc.dram_tensor("y", [B, T, V], F32, kind="ExternalOutput")

    with tile.TileContext(nc) as tc:
        import contextlib

        ctx = contextlib.ExitStack()
        with ctx:
            wp = ctx.enter_context(tc.tile_pool(name="weights", bufs=1))
            hp = ctx.enter_context(tc.tile_pool(name="hstate", bufs=1))
            cp = ctx.enter_context(tc.tile_pool(name="cstate", bufs=2))
            gp = ctx.enter_context(tc.tile_pool(name="gact", bufs=1))
            lgp = ctx.enter_context(tc.tile_pool(name="lgpool", bufs=2))
            sp = ctx.enter_context(tc.tile_pool(name="send", bufs=2))
            ap_ = ctx.enter_context(tc.tile_pool(name="amax", bufs=2))
            yp = ctx.enter_context(tc.tile_pool(name="ybuf", bufs=2))
            pg = ctx.enter_context(tc.tile_pool(name="pgates", bufs=1, space="PSUM"))
            pt = ctx.enter_context(tc.tile_pool(name="ptrans", bufs=1, space="PSUM"))
            px = ctx.enter_context(tc.tile_pool(name="pmisc", bufs=1, space="PSUM"))
            dp = ctx.enter_context(tc.tile_pool(name="dram", bufs=2, space="DRAM"))

            # ---- load weights + constants (once) ----
            w0_sb = wp.tile([128, 8, 1024], MMD, tag="w0")
            nc.sync.dma_start(out=w0_sb[:], in_=w0c[:])
            w1i_sb = wp.tile([128, 16, 1024], MMD, tag="w1i")
            nc.sync.dma_start(out=w1i_sb[:], in_=w1ic[:])
            w1h_sb = wp.tile([128, 8, 1024], MMD, tag="w1h")
            nc.sync.dma_start(out=w1h_sb[:], in_=w1hc[:])
            w0x_sb = wp.tile([1, 1024], MMD, tag="w0x")
            nc.sync.dma_start(out=w0x_sb[:], in_=w0xrep[:])
            b0_sb = wp.tile([1, 1024], MMD, tag="b0")
            nc.sync.dma_start(out=b0_sb[:], in_=b0c[:])
            b1_sb = wp.tile([1, 1024], MMD, tag="b1")
            nc.sync.dma_start(out=b1_sb[:], in_=b1c[:])
            lin_sb = wp.tile([128, 2, 64], MMD, tag="lin")
            nc.sync.dma_start(out=lin_sb[:], in_=linTc[:])
            linb8_sb = wp.tile([1, 64], MMD, tag="linb8")
            nc.sync.dma_start(out=linb8_sb[:], in_=linb8[:])
            nk_sb = wp.tile([128, 64], F32, tag="nk")
            nc.sync.dma_start(out=nk_sb[:], in_=notkeep[:])
            ident = wp.tile([128, 128], F32, tag="ident")
            make_identity(nc, ident[:])
            ones = wp.tile([1, 128], MMD, tag="ones")
            nc.sync.dma_start(out=ones[:], in_=onesrow[:])

            # ---- initial state ----
            h0T = hp.tile([128, 16, 128], MMD, tag="h0T")
            nc.sync.dma_start(out=h0T[:], in_=hT0_0[:])
            h1T = hp.tile([128, 16, 128], MMD, tag="h1T")
            nc.sync.dma_start(out=h1T[:], in_=hT0_1[:])
            cc0 = cp.tile([128, 256], F32, tag="c0")
            nc.sync.dma_start(out=cc0[:], in_=c0c[0])
            cc1 = cp.tile([128, 256], F32, tag="c1")
            nc.sync.dma_start(out=cc1[:], in_=c0c[1])
            flag_prev = ap_.tile([128, 1], F32, tag="flag")
            nc.sync.dma_start(out=flag_prev[:], in_=flag0[:])
            idxf = ap_.tile([128, 1], F32, tag="idx")
            nc.sync.dma_start(out=idxf[:], in_=idx0[:])

            def g_partials(tag, bias_sb, w_sb, hT_tile, ones_t=None,
                           close_group=False):
                """bias + w_hh partial accumulation for one layer (both dirs).
                Returns the two PSUM gate tiles [128, 512] (d0, d1)."""
                if ones_t is None:
                    ones_t = ones
                g = [pg.tile([128, 512], F32, tag=f"{tag}{d}", name=f"{tag}{d}")
                     for d in range(2)]
                for d in range(2):
                    nc.tensor.matmul(g[d][:], ones_t[:],
                                     bias_sb[:, d * 512:(d + 1) * 512],
                                     start=True, stop=False)
                for k in range(8):
                    for d in range(2):
                        nc.tensor.matmul(g[d][:], hT_tile[:, d * 8 + k, :],
                                         w_sb[:, k, d * 512:(d + 1) * 512],
                                         start=False,
                                         stop=(close_group and k == 7))
                return g

            def lstm_ew(g, cc_prev, cc_tag, sendT, per_dir=None):
                """Elementwise LSTM cell for both dirs from PSUM gates g[d];
                transposes each dir's h2 half into sendT as soon as it is
                ready. Returns (cc_new [128,256], h2 [128,256])."""
                cc_new = cp.tile([128, 256], F32, tag=cc_tag)
                h2 = gp.tile([128, 256], F32, tag=f"h2_{cc_tag}")
                for d in range(2):
                    a = gp.tile([128, 512], F32, tag=f"a{cc_tag}{d}")
                    nc.scalar.activation(a[:, 0:256], g[d][:, 0:256], AF.Sigmoid)
                    nc.scalar.activation(a[:, 384:512], g[d][:, 384:512], AF.Tanh)
                    nc.scalar.activation(a[:, 256:384], g[d][:, 256:384], AF.Sigmoid)
                    t1 = gp.tile([128, 128], F32, tag=f"t1{d}")
                    nc.vector.tensor_mul(t1[:], a[:, 128:256],
                                         cc_prev[:, d * 128:(d + 1) * 128])
                    t2 = gp.tile([128, 128], F32, tag=f"t2{d}")
                    nc.vector.tensor_mul(t2[:], a[:, 0:128], a[:, 384:512])
                    nc.vector.tensor_add(cc_new[:, d * 128:(d + 1) * 128],
                                         t1[:], t2[:])
                    tc2 = gp.tile([128, 128], F32, tag=f"tc2{d}")
                    nc.scalar.activation(tc2[:], cc_new[:, d * 128:(d + 1) * 128],
                                         AF.Tanh)
                    nc.vector.tensor_mul(h2[:, d * 128:(d + 1) * 128],
                                         a[:, 256:384], tc2[:])
                    nc.tensor.transpose(sendT[:, d * 128:(d + 1) * 128],
                                        h2[:, d * 128:(d + 1) * 128], ident[:])
                    if per_dir is not None:
                        per_dir(d)
                return cc_new, h2

            # prologue: accumulate L0 gate partials for step 0
            g0 = g_partials("g0", b0_sb, w0_sb, h0T)

            LG = None
            Lsum = None
            ybuf = None

            for t in range(T):
                # ---- argmax of logits(t-1) -> x(t) ----
                if t > 0:
                    l4 = ap_.tile([128, 4, 64], F32, tag="l4")
                    nc.vector.tensor_add(l4[:], LG[:, 0:4, :], LG[:, 4:8, :])
                    l2 = ap_.tile([128, 2, 64], F32, tag="l2")
                    nc.vector.tensor_add(l2[:], l4[:, 0:2, :], l4[:, 2:4, :])
                    Lsum = ap_.tile([128, 64], F32, tag="L")
                    nc.vector.tensor_add(Lsum[:], l2[:, 0, :], l2[:, 1, :])
                    m8 = ap_.tile([128, 8], F32, tag="m8")
                    nc.vector.max(m8[:], Lsum[:])
                    mi = ap_.tile([128, 8], U32, tag="mi")
                    nc.vector.max_index(mi[:], m8[:], Lsum[:])
                    idxf = ap_.tile([128, 1], F32, tag="idx")
                    nc.vector.tensor_copy(idxf[:], mi[:, 0:1])
                # ---- close L0 gates with x contribution ----
                x_ps = px.tile([1, 128], F32, tag="xps")
                nc.tensor.transpose(x_ps[:], idxf[:], ident[:])
                x_row = ap_.tile([1, 128], MMD, tag="xrow")
                nc.vector.tensor_copy(x_row[:], x_ps[:])
                for d in range(2):
                    nc.tensor.matmul(g0[d][:], x_row[:],
                                     w0x_sb[:, d * 512:(d + 1) * 512],
                                     start=False, stop=True)
                # ---- L0 elementwise + transpose own h0 slices ----
                sendT0 = pt.tile([128, 256], F32, tag="sendT0")
                cc0, h2_0 = lstm_ew(g0, cc0, "c0", sendT0)
                sendA = sp.tile([128, 256], MMD, tag="sendA")
                agA_in = dp.tile([128, 256], MMD, tag="agAi")
                # d0 half ships as soon as its transpose lands; the d1 copy is
                # sequenced behind it so AllGather A1 (d0) always wins the
                # collective queue ahead of A2 (d1)
                nc.vector.tensor_copy(sendA[:, 0:128], sendT0[:, 0:128])
                nc.sync.dma_start(out=agA_in[:, 0:128], in_=sendA[:, 0:128])
                pA = ap_.tile([128, 1], F32, tag="pA")
                nc.vector.tensor_scalar(pA[:], sendA[:, 0:1], 0.0, 1.0,
                                        op0=ALU.mult, op1=ALU.add)
                nc.vector.tensor_scalar(sendA[:, 128:256], sendT0[:, 128:256],
                                        pA[:], None, op0=ALU.mult)
                nc.sync.dma_start(out=agA_in[:, 128:256], in_=sendA[:, 128:256])
                # pacer: releases the g1 partial group only once the L0 send
                # path has drained, keeping those matmuls out of its way
                onesg1 = ap_.tile([1, 128], MMD, tag="onesg1")
                nc.vector.tensor_scalar(onesg1[:], sendA[0:1, 128:256], 0.0, 1.0,
                                        op0=ALU.mult, op1=ALU.add)
                # ---- flag/mask/y for step t-1 (off critical path) ----
                if t > 0:
                    flagb = ap_.tile([128, 1], F32, tag="flagb")
                    nc.vector.tensor_scalar(flagb[:], idxf[:], BLANK_IDX, None,
                                            op0=ALU.is_equal)
                    fnew = ap_.tile([128, 1], F32, tag="flag")
                    nc.vector.tensor_max(fnew[:], flag_prev[:], flagb[:])
                    flag_prev = fnew
                    tk2 = ap_.tile([128, 64], F32, tag="tk2")
                    nc.vector.scalar_tensor_tensor(tk2[:], nk_sb[:], fnew[:],
                                                   Lsum[:], op0=ALU.mult,
                                                   op1=ALU.mult)
                    s = t - 1
                    if s % CHUNK == 0:
                        ybuf = yp.tile([128, CHUNK, 64], F32, tag="ybuf")
                    nc.vector.tensor_sub(ybuf[:, s % CHUNK, :], Lsum[:], tk2[:])
                    if s % CHUNK == CHUNK - 1:
                        nc.sync.dma_start(out=y[:, s - CHUNK + 1:s + 1, :],
                                          in_=ybuf[:])
                # ---- AllGather A (h0 slices) ----
                agA_out = dp.tile([1024, 256], MMD, tag="agAo", addr_space="Shared")
                nc.gpsimd.collective_compute(
                    "AllGather", ALU.bypass, replica_groups=RG,
                    ins=[agA_in.opt()], outs=[agA_out.opt()],
                )
                # ---- L1 gate partials (run during AllGather A) ----
                g1 = g_partials("g1", b1_sb, w1h_sb, h1T, ones_t=onesg1)
                # ---- readback gathered h0T (split so matmul releases stagger;
                # alternate HWDGE (sync) and SWDGE (gpsimd) issue queues) ----
                h0T = hp.tile([128, 16, 128], MMD, tag="h0T")
                rb_eng = [0, 1, 0, 0, 1, 0, 0, 1, 0, 0, 1, 0, 0, 1, 0, 0]
                bounds = []
                lo = 0
                for g_sz in RB_GROUPS:
                    bounds.append((lo, min(16, lo + g_sz)))
                    lo += g_sz
                    if lo >= 16:
                        break
                for r, (lo, hi) in enumerate(bounds):
                    eng = nc.sync if rb_eng[r % 16] == 0 else nc.gpsimd
                    for dd in (0, 1):
                        l2_, h2_ = max(lo, dd * 8), min(hi, (dd + 1) * 8)
                        if l2_ >= h2_:
                            continue
                        eng.dma_start(
                            out=h0T[:, l2_:h2_, :],
                            in_=agA_out[(l2_ - dd * 8) * 128:(h2_ - dd * 8) * 128,
                                        dd * 128:(dd + 1) * 128].rearrange(
                                "(k p) b -> p k b", p=128),
                        )
                # ---- close L1 gates: w_ih1 over gathered h0 ----
                for kk in range(16):
                    for d in range(2):
                        nc.tensor.matmul(
                            g1[d][:], h0T[:, kk, :],
                            w1i_sb[:, kk, d * 512:(d + 1) * 512],
                            start=False, stop=(kk == 15),
                        )
                # ---- L1 elementwise + transpose own h1 slices ----
                sendT1 = pt.tile([128, 256], F32, tag="sendT1")
                cc1, h2_1 = lstm_ew(g1, cc1, "c1", sendT1)
                lpstat = sp.tile([128, 256], MMD, tag="lpstat")
                nc.vector.tensor_copy(lpstat[:], sendT1[:])
                lp = px.tile([128, 64], F32, tag="lp")
                nc.tensor.matmul(lp[:], ones[:], linb8_sb[:],
                                 start=True, stop=False)
                for d in range(2):
                    nc.tensor.matmul(lp[:], lpstat[:, d * 128:(d + 1) * 128],
                                     lin_sb[:, d, :],
                                     start=False, stop=(d == 1))
                lpc = sp.tile([128, 64], F32, tag="lpc")
                nc.vector.tensor_copy(lpc[:], lp[:])
                agB1_in = dp.tile([128, 64], F32, tag="agB1i")
                nc.sync.dma_start(out=agB1_in[:], in_=lpc[:])
                # lpc-derived all-ones column: sequences the h1 payload (and
                # next-step g0 partials) strictly after the logit send path
                p1 = ap_.tile([128, 1], F32, tag="p1")
                nc.vector.tensor_scalar(p1[:], lpc[:, 0:1], 0.0, 1.0,
                                        op0=ALU.mult, op1=ALU.add)
                sendB = sp.tile([128, 256], MMD, tag="sendB")
                nc.vector.tensor_scalar(sendB[:], lpstat[:], p1[:], None,
                                        op0=ALU.mult)
                agB2_in = dp.tile([128, 256], MMD, tag="agB2i")
                nc.sync.dma_start(out=agB2_in[:], in_=sendB[:])
                onesg0 = ap_.tile([1, 128], MMD, tag="onesg0")
                nc.vector.tensor_scalar(onesg0[:], ones[:], p1[0:1, 0:1], None,
                                        op0=ALU.mult)
                # ---- AllGather B1 (logit partials): small + first, so the
                # argmax/L0 chain overlaps the h1 AllGather B2 below ----
                agB1_out = dp.tile([1024, 64], F32, tag="agB1o",
                                   addr_space="Shared")
                nc.gpsimd.collective_compute(
                    "AllGather", ALU.bypass, replica_groups=RG,
                    ins=[agB1_in.opt()], outs=[agB1_out.opt()],
                )
                # ---- AllGather B2 (h1 slices; consumed late by g1 partials) ----
                agB2_out = dp.tile([1024, 256], MMD, tag="agB2o",
                                   addr_space="Shared")
                nc.gpsimd.collective_compute(
                    "AllGather", ALU.bypass, replica_groups=RG,
                    ins=[agB2_in.opt()], outs=[agB2_out.opt()],
                )
                # ---- readbacks: logit partials FIRST (argmax path), then h1T ----
                LG = lgp.tile([128, 8, 64], F32, tag="LG")
                nc.sync.dma_start(
                    out=LG[:],
                    in_=agB1_out.rearrange("(c p) v -> p c v", p=128),
                )
                h1T = hp.tile([128, 16, 128], MMD, tag="h1T")
                for dd in (0, 1):
                    nc.gpsimd.dma_start(
                        out=h1T[:, dd * 8:(dd + 1) * 8, :],
                        in_=agB2_out[:, dd * 128:(dd + 1) * 128].rearrange(
                            "(k p) b -> p k b", p=128),
                    )
                # ---- L0 gate partials for step t+1 (run during AllGather B) ----
                g0 = g_partials("g0", b0_sb, w0_sb, h0T, ones_t=onesg0)

            # ---- epilogue: argmax/flag/y for final step ----
            l4 = ap_.tile([128, 4, 64], F32, tag="l4")
            nc.vector.tensor_add(l4[:], LG[:, 0:4, :], LG[:, 4:8, :])
            l2 = ap_.tile([128, 2, 64], F32, tag="l2")
            nc.vector.tensor_add(l2[:], l4[:, 0:2, :], l4[:, 2:4, :])
            Lsum = ap_.tile([128, 64], F32, tag="L")
            nc.vector.tensor_add(Lsum[:], l2[:, 0, :], l2[:, 1, :])
            m8 = ap_.tile([128, 8], F32, tag="m8")
            nc.vector.max(m8[:], Lsum[:])
            mi = ap_.tile([128, 8], U32, tag="mi")
            nc.vector.max_index(mi[:], m8[:], Lsum[:])
            idxf = ap_.tile([128, 1], F32, tag="idx")
            nc.vector.tensor_copy(idxf[:], mi[:, 0:1])
            flagb = ap_.tile([128, 1], F32, tag="flagb")
            nc.vector.tensor_scalar(flagb[:], idxf[:], BLANK_IDX, None,
                                    op0=ALU.is_equal)
            fnew = ap_.tile([128, 1], F32, tag="flag")
            nc.vector.tensor_max(fnew[:], flag_prev[:], flagb[:])
            tk2 = ap_.tile([128, 64], F32, tag="tk2")
            nc.vector.scalar_tensor_tensor(tk2[:], nk_sb[:], fnew[:], Lsum[:],
                                           op0=ALU.mult, op1=ALU.mult)
            s = T - 1
            nc.vector.tensor_sub(ybuf[:, s % CHUNK, :], Lsum[:], tk2[:])
            nfin = (s % CHUNK) + 1
            nc.sync.dma_start(out=y[:, T - nfin:T, :], in_=ybuf[:, 0:nfin, :])
            # ---- final state stores ----
            nc.sync.dma_start(out=hTf_0[:], in_=h0T[:])
            nc.sync.dma_start(out=hTf_1[:], in_=h1T[:])
            nc.sync.dma_start(out=c_f[0], in_=cc0[:])
            nc.sync.dma_start(out=c_f[1], in_=cc1[:])
            nc.sync.dma_start(out=flag_f[:], in_=fnew[:])
            nc.sync.dma_start(out=idx_f[:], in_=idxf[:])
    nc.finalize()
    return nc


def prep_inputs(h0, c0, w_ih0, w_hh0, b0, w_ih1, w_hh1, b1, lin_w, lin_b):
    """Host-side packing: per-core sliced/transposed weight + state arrays."""
    h0 = np.asarray(h0, np.float32).reshape(2, 2, B, H)
    c0 = np.asarray(c0, np.float32).reshape(2, 2, B, H)
    w_ih0 = np.asarray(w_ih0, np.float32)
    w_hh0 = np.asarray(w_hh0, np.float32)
    b0 = np.asarray(b0, np.float32)
    w_ih1 = np.asarray(w_ih1, np.float32)
    w_hh1 = np.asarray(w_hh1, np.float32)
    b1 = np.asarray(b1, np.float32)
    lin_w = np.asarray(lin_w, np.float32)
    lin_b = np.asarray(lin_b, np.float32)

    linbb = np.broadcast_to(lin_b, (128, V)).copy()
    nk = np.ones((128, V), np.float32)
    nk[:, KEEP_IDX] = 0.0

    # initial transposed h state: [128, 16, 128], chunk kk = d*8+k
    def hT_init(l):
        out = np.zeros((128, 16, B), np.float32)
        for d in range(2):
            hT = h0[l, d].T.reshape(8, 128, B)          # [k, p, b]
            out[:, d * 8:(d + 1) * 8, :] = hT.transpose(1, 0, 2)
        return out

    hT0_0 = hT_init(0)
    hT0_1 = hT_init(1)

    in_maps = []
    for c in range(NCORES):
        rows = np.concatenate([np.arange(gb + c * 128, gb + c * 128 + 128)
                               for gb in GBASE])

        def packT(w, kt):
            # w: (4H, K*128) -> select rows -> [p, k, n]
            sel = w[rows, :]  # (512, kt*128)
            return np.ascontiguousarray(
                sel.reshape(512, kt, 128).transpose(2, 1, 0))

        w0c = np.concatenate([packT(w_hh0[d], 8) for d in range(2)], axis=2)
        w1ic = np.concatenate([packT(w_ih1[d], 16) for d in range(2)], axis=2)
        w1hc = np.concatenate([packT(w_hh1[d], 8) for d in range(2)], axis=2)
        w0xr = np.concatenate([w_ih0[0][rows, 0],
                               w_ih0[1][rows, 0]])[None, :]
        b0row = np.concatenate([b0[0][rows], b0[1][rows]])[None, :]
        b1row = np.concatenate([b1[0][rows], b1[1][rows]])[None, :]
        linTc = np.stack(
            [lin_w[:, c * 128:(c + 1) * 128].T,
             lin_w[:, H + c * 128:H + (c + 1) * 128].T], axis=1)
        c0slice = np.zeros((2, 128, 256), np.float32)
        for l in range(2):
            for d in range(2):
                c0slice[l, :, d * 128:(d + 1) * 128] = \
                    c0[l, d][:, c * 128:(c + 1) * 128]
        in_maps.append({
            "w0c": tf32_round(np.ascontiguousarray(w0c)),
            "w1ic": tf32_round(np.ascontiguousarray(w1ic)),
            "w1hc": tf32_round(np.ascontiguousarray(w1hc)),
            "w0xrep": tf32_round(np.ascontiguousarray(w0xr)),
            "b0c": tf32_round(b0row),
            "b1c": tf32_round(b1row),
            "linTc": tf32_round(np.ascontiguousarray(linTc)),
            "linb8": (lin_b / 8.0).reshape(1, V).astype(np.float32),
            "notkeep": nk,
            "hT0_0": tf32_round(hT0_0),
            "hT0_1": tf32_round(hT0_1),
            "c0c": np.ascontiguousarray(c0slice),
            "onesrow": np.ones((1, 128), np.float32),
            "idx0": np.full((128, 1), MASK_IDX, np.float32),
            "flag0": np.zeros((128, 1), np.float32),
        })
    return in_maps


_NC_CACHE = {}


def _get_nc(T):
    if T not in _NC_CACHE:
        _NC_CACHE[T] = build(T)
    return _NC_CACHE[T]


T_LAUNCH = 256


def kernel(h0, c0, w_ih0, w_hh0, b0, w_ih1, w_hh1, b1, lin_w, lin_b,
           decoder_output_length, batch_size, _want_results=False):
    T = int(decoder_output_length)
    assert int(batch_size) == B
    in_maps = prep_inputs(h0, c0, w_ih0, w_hh0, b0, w_ih1, w_hh1, b1,
                          lin_w, lin_b)
    chunks = []
    t_done = 0
    res = None
    while t_done < T:
        t_this = min(T_LAUNCH, T - t_done)
        nc = _get_nc(t_this)
        res = bass_utils.run_bass_kernel_spmd(nc, in_maps,
                                              core_ids=list(range(NCORES)))
        chunks.append(res.results[0]["y"])
        t_done += t_this
        if t_done < T:
            idxs = res.results[0]["idx_f"]  # (128,1) float indices
            for c in range(NCORES):
                rc = res.results[c]
                in_maps[c] = dict(in_maps[c])
                in_maps[c]["hT0_0"] = rc["hTf_0"]
                in_maps[c]["hT0_1"] = rc["hTf_1"]
                in_maps[c]["c0c"] = rc["c_f"]
                in_maps[c]["flag0"] = rc["flag_f"]
                in_maps[c]["idx0"] = np.ascontiguousarray(idxs)
    out = np.concatenate(chunks, axis=1) if len(chunks) > 1 else chunks[0]
    if _want_results:
        return out, res
    return out
